# revision 1
# baseline (speedup 1.0000x reference)
"""Trainium2 Bass kernel for nn_DeepTensorNN (gnn_message_passing).

Reference math (B=64, N=256, E=20 atom-emb dims, F=25 RBF centers):
    mask  = (z != 0)
    cfeat = emb[z] * mask                              [B,N,20]
    dfeat = exp(-(dist[...,None]-mu)^2 / (2*0.5^2))    [B,N,N,25]
    msg   = tanh(cfeat@Vw1.T + dfeat@Vw2.T + Vb) * mask_i
    agg   = msg.sum(j); c = cfeat + agg
    out_b = sum_i ( tanh(c) @ W1.T + b1 ) @ W2.T + b2

Device strategy (data-parallel over batch, 8 b's per core):
  * -2(d-mu)^2 = 4mu*d - 2d^2 - 2mu^2 is affine in (d, d^2), so a small
    PE matmul builds the exponent for 25 RBF centers x 5 atoms = 125
    partitions at once; exp's per-partition bias adds -2mu^2. For full
    bf16 matmul speed without losing exponent precision, d and d^2 are
    split on the host into bf16 hi+lo parts and the 4mu weights into
    bf16 hi+lo parts: E = wh*dh + wl*dh + wh*dl - 2(d2h + d2l), giving
    |dE| <= ~1.2e-3 (the dropped wl*dl term and bf16 residuals).
  * The per-(b,i) bias A[b,i]+Vb is folded into the 25->20 RBF matmul
    via constant one-hot rows (K=125+2+pad=128), so both ACT passes
    (exp, tanh) run one instruction per 2048 pair-columns.
  * DVE tensor_reduce sums tanh outputs over the 256 neighbors.
  * Host (numpy) does the cheap parts: emb[z] gather, A=cfeat@Vw1.T+Vb,
    dist^2 and the bf16 splits, and the final tiny MLP + reductions.

ACT (ScalarE) is the bottleneck: 104 ACTIVATEs ~= 204us per core.
"""

import os
from contextlib import ExitStack

import ml_dtypes
import numpy as np

import concourse.bacc as bacc
import concourse.mybir as mybir
import concourse.tile as tile
from concourse.bass_utils import run_bass_kernel_spmd

# ----------------------------------------------------------------------------
# Problem constants (hardcoded; kernel.py must be self-contained)
B, N = 64, 256
ATOMEMB = 20
N_CORES = 8
BPC = B // N_CORES          # batches per core = 8
NSUPER = 4                  # supertiles per core: 2 b-groups x 2 halves
NBATCH = 13                 # matmul/ACT batches per supertile
BLK_COLS = 256              # j columns per block
NBLK = 26                   # i-blocks per (b, half): 25 + 1 overlap block
NCOMP = 5                   # exponent components: dh(wh), dh(wl), dl, d2h, d2l

F32 = mybir.dt.float32
BF16 = mybir.dt.float16    # fp16: same PE rate as bf16, 4x finer mantissa
NP_BF16 = np.float16

_MUS = np.arange(0.0, 5.0, 0.2, dtype=np.float32)  # [25]


def _row_of(k: int, q: int) -> int:
    """i-row (within a 128-row half) of stack-position q in block k."""
    return 5 * k + q if k <= 24 else 123 + q


def _slot_gk(beta: int, j: int):
    """column-slot j of batch beta -> (b-slot g, block k)."""
    return j // 2, 2 * beta + (j % 2)


# ----------------------------------------------------------------------------
# Host-side constant tensors (shared by all cores)

def _build_consts():
    mus4 = 4.0 * _MUS
    wh = mus4.astype(NP_BF16).astype(np.float32)
    wl = (mus4 - wh).astype(NP_BF16).astype(np.float32)
    comp_w = [wh, wl, wh, np.full(25, -2.0, np.float32),
              np.full(25, -2.0, np.float32)]
    # sel[32g + 5r + q, 25q' + f] = (q==q') * comp_w[r][f]
    sel = np.zeros((121, 125), dtype=np.float32)
    for g in range(4):
        for r in range(NCOMP):
            for q in range(5):
                sel[32 * g + 5 * r + q, 25 * q:25 * q + 25] = comp_w[r]
    # exp bias: -2*mu_f^2 per partition p = 25q+f
    mu2 = np.tile(-2.0 * _MUS * _MUS, 5).astype(np.float32).reshape(125, 1)
    # two all-ones rows appended to the RBF rhs; with per-slot RBF matmuls
    # they carry the bf16 hi (row 125) and lo (row 126) parts of the bias
    onehot = np.ones((2, 8 * BLK_COLS), dtype=np.float32)
    return (sel.astype(NP_BF16), mu2, onehot.astype(NP_BF16))


def _build_blockdiag(Vw2: np.ndarray) -> np.ndarray:
    # blockdiag[25q+f, 100j + 20q'+o] = (q==q') * Vw2[o, f] for the eight
    # column-slots j (one N=256 RBF matmul per slot).
    bd = np.zeros((125, 800), dtype=np.float32)
    for j in range(8):
        for q in range(5):
            bd[25 * q:25 * q + 25, 100 * j + 20 * q:100 * j + 20 * q + 20] = Vw2.T
    return bd.astype(NP_BF16)


def _build_biasrows(Abias_core: np.ndarray) -> np.ndarray:
    """Abias_core: [BPC, 256, 20] -> biasrows [52, 2, 800] (bf16).

    biasrows[13s+beta, v, 100j + 20q + o]: bias of the block at
    column-slot j; v=0 its bf16 hi part (lhsT row 125), v=1 the lo
    residual (row 126). hi+lo is exact to ~2^-17 relative.
    """
    full = np.zeros((NSUPER * NBATCH, 800), dtype=np.float32)
    for s in range(NSUPER):
        G, h = s // 2, s % 2
        for beta in range(NBATCH):
            for j in range(8):
                g, k = _slot_gk(beta, j)
                b_local = 4 * G + g
                for q in range(5):
                    i = 128 * h + _row_of(k, q)
                    full[NBATCH * s + beta,
                         100 * j + 20 * q:100 * j + 20 * q + 20] = \
                        Abias_core[b_local, i]
    hi16 = full.astype(NP_BF16)
    lo16 = (full - hi16.astype(np.float32)).astype(NP_BF16)
    out = np.zeros((NSUPER * NBATCH, 2, 800), dtype=NP_BF16)
    out[:, 0] = hi16
    out[:, 1] = lo16
    return out


def _build_output_index():
    """Index arrays mapping device output [NSUPER,100,104] -> agg[b_local,i].

    Returns (B_IDX, I_IDX) of shape [NSUPER, 104, 5].
    """
    b_idx = np.zeros((NSUPER, 104, 5), dtype=np.int64)
    i_idx = np.zeros((NSUPER, 104, 5), dtype=np.int64)
    for s in range(NSUPER):
        G, h = s // 2, s % 2
        for beta in range(NBATCH):
            for j in range(8):
                g, k = _slot_gk(beta, j)
                col = 8 * beta + j
                for q in range(5):
                    b_idx[s, col, q] = 4 * G + g
                    i_idx[s, col, q] = 128 * h + _row_of(k, q)
    return b_idx, i_idx


_B_IDX, _I_IDX = _build_output_index()


def make_in_maps(z, dist, emb, Vw, Vb):
    """Host prep: per-core input dicts for the device program."""
    mask = (z != 0).astype(np.float32)
    emb0 = emb.copy()
    emb0[0] = 0.0
    cfeat = emb0[z]                                          # [B,N,20]
    Vw1, Vw2 = Vw[:, :ATOMEMB], Vw[:, ATOMEMB:]
    Abias = cfeat @ Vw1.T + Vb                               # [B,N,20]

    # bf16 hi/lo splits of d and d^2 (component rows: dh(wh), dh(wl), dl,
    # d2h, d2l), pre-arranged on the host into the exact SBUF layout of the
    # per-supertile P tile [121, 26*256] so one contiguous DMA loads it:
    # P[s, 32g + 5r + q, 256k + j] = comp_r[4G+g, 128h + row(k,q), j]
    dh16 = dist.astype(NP_BF16)
    dh = dh16.astype(np.float32)
    dl16 = (dist - dh).astype(NP_BF16)
    d2 = dist * dist
    d2h16 = d2.astype(NP_BF16)
    d2h = d2h16.astype(np.float32)
    d2l16 = (d2 - d2h).astype(NP_BF16)
    comp = np.stack([dh16, dl16, d2h16, d2l16], axis=1)      # [B,4,N,N]
    rows_kq = np.array([[_row_of(k, q) for k in range(NBLK)]
                        for q in range(5)])                  # [5, NBLK]
    COMP_PLANE = (0, 0, 1, 2, 3)
    pcomp = np.zeros((B, NSUPER, 121, NBLK, N), dtype=NP_BF16)
    for s_ in range(NSUPER):
        G, h = s_ // 2, s_ % 2
        for g in range(4):
            for r in range(5):
                for q in range(5):
                    pcomp[:, s_, 32 * g + 5 * r + q] = \
                        comp[:, COMP_PLANE[r], 128 * h + rows_kq[q]]
    # select the 4 b's of each supertile's b-group
    bsel = np.array([[4 * (s_ // 2) + g for g in range(4)]
                     for s_ in range(NSUPER)])               # [NSUPER, 4]

    sel, mu2, onehot = _build_consts()
    blockdiag = _build_blockdiag(Vw2)

    in_maps = []
    for c in range(N_CORES):
        bsl = slice(BPC * c, BPC * (c + 1))
        pc_core = pcomp[bsl]                                 # [BPC,NSUPER,...]
        ptiles = np.zeros((NSUPER, 121, NBLK * N), dtype=NP_BF16)
        for s_ in range(NSUPER):
            for g in range(4):
                gsl = slice(32 * g, 32 * g + 25)
                ptiles[s_, gsl] = pc_core[bsel[s_, g], s_, gsl].reshape(
                    25, NBLK * N)
        in_maps.append({
            "pcomp": ptiles,
            "biasrows": _build_biasrows(Abias[bsl]),
            "blockdiag": blockdiag,
            "onehot": onehot,
            "sel": sel,
            "mu2": mu2,
        })
    return in_maps, cfeat, mask


# ----------------------------------------------------------------------------
# Device program

def build_program():
    nc = bacc.Bacc("TRN2", target_bir_lowering=False, debug=False,
                   enable_asserts=True, num_devices=N_CORES)
    Exp = mybir.ActivationFunctionType.Exp
    Tanh = mybir.ActivationFunctionType.Tanh

    pcomp_d = nc.dram_tensor("pcomp", [NSUPER, 121, NBLK * N], BF16,
                             kind="ExternalInput")
    biasrows_d = nc.dram_tensor("biasrows", [NSUPER * NBATCH, 2, 800], BF16,
                                kind="ExternalInput")
    blockdiag_d = nc.dram_tensor("blockdiag", [125, 800], BF16,
                                 kind="ExternalInput")
    onehot_d = nc.dram_tensor("onehot", [2, 2048], BF16, kind="ExternalInput")
    sel_d = nc.dram_tensor("sel", [121, 125], BF16, kind="ExternalInput")
    mu2_d = nc.dram_tensor("mu2", [125, 1], F32, kind="ExternalInput")
    agg_d = nc.dram_tensor("aggout", [NSUPER, 100, 104], F32,
                           kind="ExternalOutput")

    with tile.TileContext(nc) as tc, ExitStack() as ctx:
        const_pool = ctx.enter_context(tc.tile_pool(name="const", bufs=1))
        p_pool = ctx.enter_context(tc.tile_pool(name="pd", bufs=2))
        rhs_pool = ctx.enter_context(tc.tile_pool(name="rhs", bufs=3))
        msg_pool = ctx.enter_context(tc.tile_pool(name="msg", bufs=6))
        lhst_pool = ctx.enter_context(tc.tile_pool(name="lhst", bufs=1))
        aggo_pool = ctx.enter_context(tc.tile_pool(name="aggo", bufs=2))
        psum_pool = ctx.enter_context(
            tc.tile_pool(name="ps", bufs=2, space="PSUM"))

        sel_t = const_pool.tile([121, 125], BF16)
        nc.sync.dma_start(sel_t[:], sel_d.ap())
        mu2_t = const_pool.tile([125, 1], F32)
        nc.sync.dma_start(mu2_t[:], mu2_d.ap())

        # persistent RBF lhsT tiles (2 manual double-buffer), hi part in
        # cols 0:400 and lo part in cols 400:800; rows 0-124 constant Vw2
        # blockdiag, rows 125-126 rewritten with the per-batch bias
        lhsT_t = [lhst_pool.tile([127, 800], BF16, tag=f"lh{i}",
                                 name=f"lh{i}") for i in range(2)]
        for t in lhsT_t:
            nc.sync.dma_start(t[0:125, :], blockdiag_d.ap())

        # persistent rhs tiles (rotated): one-hot bias rows loaded once,
        # rows 0-124 rewritten by exp each batch
        rhs_tiles = [rhs_pool.tile([127, 2048], BF16, tag=f"rh{i}",
                                   name=f"rh{i}") for i in range(3)]
        for t in rhs_tiles:
            nc.sync.dma_start(t[125:127, :], onehot_d.ap())
        # Software pipeline (one-batch lag for tanh+reduce) so the ACT
        # stream is exp_{k+1}, tanh_k, ... — the RBF matmuls of batch k run
        # under exp_{k+1} instead of sitting between exp_k and tanh_k.
        agg_tiles = {}
        pending = None  # (ps, s, beta)

        def finish(p):
            ps, s_, beta_ = p
            msg_t = msg_pool.tile([100, 2048], F32, name="msg_t")
            nc.scalar.activation(msg_t[:], ps[0:100, :], Tanh)
            nc.vector.tensor_reduce(
                agg_tiles[s_][:, 8 * beta_:8 * beta_ + 8],
                msg_t[:].rearrange("p (c j) -> p c j", j=BLK_COLS),
                axis=mybir.AxisListType.X, op=mybir.AluOpType.add)
            if beta_ == NBATCH - 1:
                nc.sync.dma_start(agg_d.ap()[s_], agg_tiles[s_][:])

        bi = 0
        last_rbf = None
        for s in range(NSUPER):
            G, h = s // 2, s % 2
            P_t = p_pool.tile([121, NBLK * BLK_COLS], BF16)
            for cc in range(NBATCH):
                nc.sync.dma_start(P_t[:, 512 * cc:512 * cc + 512],
                                  pcomp_d.ap()[s, :, 512 * cc:512 * cc + 512])

            agg_tiles[s] = aggo_pool.tile([100, 104], F32, name="agg_t")
            for beta in range(NBATCH):
                lt = lhsT_t[bi % 2]
                rhs_t = rhs_tiles[bi % 3]
                bi += 1
                nc.sync.dma_start(lt[125:127, :],
                                  biasrows_d.ap()[NBATCH * s + beta])

                ps = psum_pool.tile([125, 2048], F32, name="ps")

                # exponent matmuls: one K=25, N=512 matmul per 32-row
                # group g, covering column-slots 2g and 2g+1 (PSUM bank g);
                # the four run concurrently on disjoint PE sub-arrays. Pin
                # them behind the previous batch's RBF matmuls so a PSUM
                # slot wait cannot head-of-line-block the strict-FIFO PE.
                for g in range(4):
                    k0 = 2 * beta
                    mm = nc.tensor.matmul(
                        ps[0:125, 512 * g:512 * g + 512],
                        sel_t[32 * g:32 * g + 25, :],
                        P_t[32 * g:32 * g + 25,
                            BLK_COLS * k0:BLK_COLS * (k0 + 2)],
                        start=True, stop=True, tile_position=(32 * g, 0))
                    if last_rbf is not None:
                        tile.add_dep_helper(mm.ins, last_rbf.ins, sync=False,
                                            reason="PE order: E after prev rbf")

                nc.scalar.activation(rhs_t[0:125, :], ps[0:125, :], Exp,
                                     bias=mu2_t[:, 0:1], scale=1.0)

                if pending is not None:
                    finish(pending)

                # 25->20 RBF matmuls, one N=256 matmul per column-slot;
                # rows 125/126 of the rhs are all-ones and add the bf16
                # hi/lo split bias from lhsT rows 125/126 (exact bias)
                for j in range(8):
                    last_rbf = nc.tensor.matmul(
                        ps[0:100, BLK_COLS * j:BLK_COLS * (j + 1)],
                        lt[:, 100 * j:100 * j + 100],
                        rhs_t[:, BLK_COLS * j:BLK_COLS * (j + 1)],
                        start=True, stop=True)

                pending = (ps, s, beta)

        finish(pending)

    nc.compile()
    return nc


_NC_CACHE = None


def _get_program():
    global _NC_CACHE
    if _NC_CACHE is None:
        _NC_CACHE = build_program()
    return _NC_CACHE


# ----------------------------------------------------------------------------
# Public entry point

LAST_RESULT = None  # test harness reads exec_time_ns from here


def kernel(z, dist, emb, Vw, Vb, W1, b1, W2, b2):
    z = np.asarray(z)
    dist = np.asarray(dist, dtype=np.float32)
    emb = np.asarray(emb, dtype=np.float32)
    Vw = np.asarray(Vw, dtype=np.float32)
    Vb = np.asarray(Vb, dtype=np.float32)
    W1 = np.asarray(W1, dtype=np.float32)
    b1 = np.asarray(b1, dtype=np.float32)
    W2 = np.asarray(W2, dtype=np.float32)
    b2 = np.asarray(b2, dtype=np.float32)

    in_maps, cfeat, mask = make_in_maps(z, dist, emb, Vw, Vb)

    nc = _get_program()
    res = run_bass_kernel_spmd(nc, in_maps, core_ids=list(range(N_CORES)))
    global LAST_RESULT
    LAST_RESULT = res

    # assemble agg[b, i, o] from per-core outputs [NSUPER, 100, 104]
    agg = np.zeros((B, N, ATOMEMB), dtype=np.float32)
    for c in range(N_CORES):
        v = res.results[c]["aggout"].reshape(NSUPER, 5, 20, 104)
        v = v.transpose(0, 3, 1, 2)                         # [s, col, q, o]
        agg[BPC * c + _B_IDX, _I_IDX] = v

    # tail MLP on host
    cf = cfeat + mask[..., None] * agg                      # [B,N,20]
    hdn = np.tanh(cf) @ W1.T + b1                           # [B,N,10]
    e = hdn @ W2.T + b2                                     # [B,N,1]
    return e.sum(axis=1)[:, 0].astype(np.float32)           # [B]



# revision 2
# speedup vs baseline: 3.9557x; 3.9557x over previous
"""Trainium2 Bass kernel for nn_DeepTensorNN (gnn_message_passing).

Reference math (B=64, N=256, E=20 atom-emb dims, 25 RBF centers):
    mask  = (z != 0)
    cfeat = emb[z] * mask                              [B,N,20]
    dfeat = exp(-2 (dist-mu)^2)                        [B,N,N,25]
    msg   = tanh(cfeat@Vw1.T + dfeat@Vw2.T + Vb) * mask_i
    agg   = msg.sum(j); c = cfeat + agg
    out_b = sum_i ( tanh(c) @ W1.T + b1 ) @ W2.T + b2

Algorithmic restructure (device does only the O(N^2) part):
  With A = cfeat@Vw1.T + Vb and phi_o(d) = sum_f Vw2[o,f] exp(-2(d-mu_f)^2),
  the per-pair argument x = A + phi_o(d) stays small (|x| < ~0.85), so
  tanh(x) is replaced by an odd polynomial p(x) = c1 x + c3 x^3 + c5 x^5
  (LSQ fit on the actual range, max err ~3e-4).  Then
      sum_j p(A + phi) = sum_{m=0..5} q_m(A) * S_m,   S_m = sum_j phi^m(d_j),
  and each phi_o^m(d) is a smooth 1-D function of d, refit on the host in a
  16-Gaussian basis psi_f(d) = exp(-GAMMA (d - t_f)^2):
      S_m(b,i,o) = sum_f Wm[m][o,f] * G_f(b,i),  G_f(b,i) = sum_j psi_f(d_bij).
  The device therefore only computes G: a Gaussian-RBF expansion of dist
  plus a sum over neighbors j.  All tanh / per-pair matmul work vanishes;
  q_m, Wm fits and the final combine are cheap per-(b,i) host numpy.

Device pipeline per 2048-column batch (128 partitions = 8 atoms x 16 f):
  * PE: 4 matmuls (2 PE bands x 2 PSUM banks) build the exponent
    -GAMMA d^2 + 2 GAMMA t_f d from fp16 hi/lo splits of d and d^2
    (K=40 rows: 5 components x 8 atoms), into PSUM [128, 2048].
  * ACT: one EXP over [128, 2048] with per-partition bias -GAMMA t_f^2,
    writing fp16 psi to SBUF.  ACT is the bottleneck: 32 EXPs/core.
  * DVE: two fp16 2x-mode tree folds (256->64 per j-run) + one
    tensor_reduce to f32 gives the 8 per-slot j-sums.
Data-parallel over batch: core c handles b in [8c, 8c+8), as 4 supertiles
of 2 b's; G returned as [4, 128, 64] f32 per core.
"""

import os
from contextlib import ExitStack

import numpy as np

import concourse.bacc as bacc
import concourse.mybir as mybir
import concourse.tile as tile
from concourse.bass_utils import run_bass_kernel_spmd

# ----------------------------------------------------------------------------
# Problem constants (hardcoded; kernel.py must be self-contained)
B, N = 64, 256
ATOMEMB = 20
N_CORES = 8
BPC = B // N_CORES          # batches per core = 8
NSUPER = 4                  # supertiles per core (2 b's each)
NBATCH = 8                  # 2048-col batches per supertile
NCOMP = 5                   # exponent rows: dh(wh), dh(wl), dl(wh), d2h, d2l
NF = 16                     # Gaussian basis size
NQ = 8                      # atoms packed per column
GAMMA = 4.5                 # basis exp(-GAMMA (d-t)^2); exactly fp16
T_CENTERS = np.linspace(-0.1, 5.1, NF)
MDEG = 5                    # odd-poly degree for tanh

F32 = mybir.dt.float32
FP16 = mybir.dt.float16
NP_FP16 = np.float16

_REF_MUS = np.arange(0.0, 5.0, 0.2)   # reference's 25 RBF centers


# ----------------------------------------------------------------------------
# Host-side constant tensors (shared by all cores)

def _build_sel():
    """sel[64*band + 8r + q, 16q' + f] = (q==q') * w_r[f], fp16 [128,128]."""
    beta = 2.0 * GAMMA * T_CENTERS
    wh = beta.astype(NP_FP16).astype(np.float64)
    wl = (beta - wh).astype(NP_FP16).astype(np.float64)
    gam = np.full(NF, -GAMMA)
    comp_w = [wh, wl, wh, gam, gam]
    sel = np.zeros((128, 128), dtype=np.float32)
    for band in range(2):
        for r in range(NCOMP):
            for q in range(NQ):
                sel[64 * band + 8 * r + q, 16 * q:16 * q + 16] = comp_w[r]
    return sel.astype(NP_FP16)


def _build_mu2():
    ct = (-GAMMA * T_CENTERS * T_CENTERS).astype(np.float32)
    return np.tile(ct, NQ).reshape(128, 1)


def make_in_maps(dist):
    """Host prep: per-core input dicts (pcomp layout) for the device."""
    dist = dist.astype(np.float32)
    dh16 = dist.astype(NP_FP16)
    dl16 = (dist - dh16.astype(np.float32)).astype(NP_FP16)
    d2 = dist * dist
    d2h16 = d2.astype(NP_FP16)
    d2l16 = (d2 - d2h16.astype(np.float32)).astype(NP_FP16)
    comp = (dh16, dh16, dl16, d2h16, d2l16)   # per component row r

    sel = _build_sel()
    mu2 = _build_mu2()

    in_maps = []
    for c in range(N_CORES):
        pcomp = np.zeros((NSUPER, 128, 32 * N), dtype=NP_FP16)
        for st in range(NSUPER):
            for band in range(2):
                b = 8 * c + 2 * st + band
                for r in range(NCOMP):
                    # row q, col 256k+j <- comp[r][b][8k+q, j]
                    blk = comp[r][b].reshape(32, NQ, N).transpose(1, 0, 2)
                    pcomp[st, 64 * band + 8 * r:64 * band + 8 * r + 8] = \
                        blk.reshape(NQ, 32 * N)
        in_maps.append({"pcomp": pcomp, "sel": sel, "mu2": mu2})
    return in_maps


# ----------------------------------------------------------------------------
# Device program

def build_program():
    nc = bacc.Bacc("TRN2", target_bir_lowering=False, debug=False,
                   enable_asserts=True, num_devices=N_CORES)
    Exp = mybir.ActivationFunctionType.Exp

    pcomp_d = nc.dram_tensor("pcomp", [NSUPER, 128, 32 * N], FP16,
                             kind="ExternalInput")
    sel_d = nc.dram_tensor("sel", [128, 128], FP16, kind="ExternalInput")
    mu2_d = nc.dram_tensor("mu2", [128, 1], F32, kind="ExternalInput")
    g_d = nc.dram_tensor("gout", [NSUPER, 128, 8 * NBATCH], F32,
                         kind="ExternalOutput")

    with tile.TileContext(nc) as tc, ExitStack() as ctx:
        const_pool = ctx.enter_context(tc.tile_pool(name="const", bufs=1))
        p_pool = ctx.enter_context(tc.tile_pool(name="pd", bufs=2))
        psi_pool = ctx.enter_context(tc.tile_pool(name="psi", bufs=3))
        f1_pool = ctx.enter_context(tc.tile_pool(name="f1", bufs=2))
        f2_pool = ctx.enter_context(tc.tile_pool(name="f2", bufs=2))
        g_pool = ctx.enter_context(tc.tile_pool(name="g", bufs=2))
        psum_pool = ctx.enter_context(
            tc.tile_pool(name="ps", bufs=2, space="PSUM"))

        sel_t = const_pool.tile([128, 128], FP16)
        nc.sync.dma_start(sel_t[:], sel_d.ap())
        mu2_t = const_pool.tile([128, 1], F32)
        nc.sync.dma_start(mu2_t[:], mu2_d.ap())

        for st in range(NSUPER):
            P_t = p_pool.tile([128, 32 * N], FP16)
            for band in range(2):
                r0 = 64 * band
                for ch in range(4):
                    nc.sync.dma_start(
                        P_t[r0:r0 + 40, 2048 * ch:2048 * ch + 2048],
                        pcomp_d.ap()[st, r0:r0 + 40,
                                     2048 * ch:2048 * ch + 2048])

            G_t = g_pool.tile([128, 8 * NBATCH], F32, name="G_t")
            for c in range(NBATCH):
                ps = psum_pool.tile([128, 2048], F32, name="ps")
                for band in range(2):
                    r0 = 64 * band
                    for h in range(2):
                        d0 = 1024 * band + 512 * h
                        s0 = 1024 * c + 512 * h
                        nc.tensor.matmul(
                            ps[0:128, d0:d0 + 512],
                            sel_t[r0:r0 + 40, :],
                            P_t[r0:r0 + 40, s0:s0 + 512],
                            start=True, stop=True, tile_position=(r0, 0))

                psi_t = psi_pool.tile([128, 2048], FP16, name="psi_t")
                nc.scalar.activation(psi_t[:], ps[0:128, :], Exp,
                                     bias=mu2_t[:, 0:1], scale=1.0)

                f1_t = f1_pool.tile([128, 1024], FP16, name="f1_t")
                v = psi_t[:].rearrange("p (s j) -> p s j", j=256)
                with nc.allow_low_precision(reason="fp16 tree fold"):
                    nc.vector.tensor_tensor(
                        f1_t[:].rearrange("p (s j) -> p s j", j=128),
                        v[:, :, 0:128], v[:, :, 128:256],
                        op=mybir.AluOpType.add)
                    f2_t = f2_pool.tile([128, 512], FP16, name="f2_t")
                    v1 = f1_t[:].rearrange("p (s j) -> p s j", j=128)
                    nc.vector.tensor_tensor(
                        f2_t[:].rearrange("p (s j) -> p s j", j=64),
                        v1[:, :, 0:64], v1[:, :, 64:128],
                        op=mybir.AluOpType.add)
                nc.vector.tensor_reduce(
                    G_t[:, 8 * c:8 * c + 8],
                    f2_t[:].rearrange("p (s j) -> p s j", j=64),
                    axis=mybir.AxisListType.X, op=mybir.AluOpType.add)

            nc.sync.dma_start(g_d.ap()[st], G_t[:])

    nc.compile()
    return nc


_NC_CACHE = None


def _get_program():
    global _NC_CACHE
    if _NC_CACHE is None:
        _NC_CACHE = build_program()
    return _NC_CACHE


# ----------------------------------------------------------------------------
# Host-side math: tanh polynomial + basis refits (input-dependent, cheap)

def _host_fits(A, Vw2):
    grid = np.linspace(0.0, 5.0, 2501)
    phi_grid = np.exp(-2.0 * (grid[:, None] - _REF_MUS) ** 2) @ Vw2.T
    R = np.abs(A).max() + np.abs(phi_grid).max() + 1e-3

    x = np.linspace(-R, R, 4001)
    X = np.stack([x, x ** 3, x ** 5], 1)
    (c1, c3, c5), *_ = np.linalg.lstsq(X, np.tanh(x), rcond=None)

    PSI = np.exp(-GAMMA * (grid[:, None] - T_CENTERS) ** 2)
    Gm = PSI.T @ PSI + 1e-7 * np.eye(NF)
    Wm = [None]
    for m in range(1, MDEG + 1):
        Wm.append(np.linalg.solve(Gm, PSI.T @ (phi_grid ** m)).T)

    q = [c1 * A + c3 * A ** 3 + c5 * A ** 5,
         c1 + 3 * c3 * A ** 2 + 5 * c5 * A ** 4,
         3 * c3 * A + 10 * c5 * A ** 3,
         c3 + 10 * c5 * A ** 2,
         5 * c5 * A,
         np.full_like(A, c5)]
    return q, Wm


def _assemble_g(results):
    """Per-core gout [NSUPER,128,64] -> G[b, i, f] full [B,N,NF]."""
    G = np.zeros((B, N, NF), dtype=np.float32)
    for c in range(N_CORES):
        R_c = results[c]["gout"]                       # [4, 128, 64]
        T = R_c.reshape(NSUPER, NQ, NF, NBATCH, 2, 4)  # st,q,f,c,band,s4
        G[8 * c:8 * c + 8] = T.transpose(0, 4, 3, 5, 1, 2).reshape(8, N, NF)
    return G


# ----------------------------------------------------------------------------
# Public entry point

LAST_RESULT = None  # test harness reads exec_time_ns from here


def kernel(z, dist, emb, Vw, Vb, W1, b1, W2, b2):
    z = np.asarray(z)
    dist = np.asarray(dist, dtype=np.float32)
    emb = np.asarray(emb, dtype=np.float32)
    Vw = np.asarray(Vw, dtype=np.float32)
    Vb = np.asarray(Vb, dtype=np.float32)
    W1 = np.asarray(W1, dtype=np.float32)
    b1 = np.asarray(b1, dtype=np.float32)
    W2 = np.asarray(W2, dtype=np.float32)
    b2 = np.asarray(b2, dtype=np.float32)

    mask = (z != 0).astype(np.float32)
    emb0 = emb.copy()
    emb0[0] = 0.0
    cfeat = emb0[z]                                       # [B,N,20]
    Vw1, Vw2 = Vw[:, :ATOMEMB], Vw[:, ATOMEMB:]
    A = (cfeat @ Vw1.T + Vb).astype(np.float64)           # [B,N,20]

    in_maps = make_in_maps(dist)
    nc = _get_program()
    res = run_bass_kernel_spmd(nc, in_maps, core_ids=list(range(N_CORES)))
    global LAST_RESULT
    LAST_RESULT = res

    G = _assemble_g(res.results).astype(np.float64)       # [B,N,16]

    q, Wm = _host_fits(A, Vw2.astype(np.float64))
    agg = q[0] * float(N)
    for m in range(1, MDEG + 1):
        agg = agg + q[m] * (G @ Wm[m].T)

    cf = cfeat + mask[..., None] * agg                    # [B,N,20]
    hdn = np.tanh(cf) @ W1.T + b1
    e = hdn @ W2.T + b2
    return e.sum(axis=1)[:, 0].astype(np.float32)         # [B]
